# revision 26
# baseline (speedup 1.0000x reference)
"""BitNet attention layer (quantized QKV + attention + quantized dense + LN)
as a Bass/Tile SPMD kernel for 8 Trainium2 NeuronCores.

Sharding: core c = 2*b + g handles batch b (of 4) and head-group g (of 2,
8 heads each).  QKV projection + attention are fully local per core
(tensor-parallel over heads, data-parallel over batch); the dense output
projection is tensor-parallel over its input dim, pair-reduced with a
token-interleaved chunked ReduceScatter that pipelines with the dense
matmuls, so each core finishes residual+layernorm on its half of the
batch's tokens.  Cross-core scalars (weight abs-means, activation
abs-maxes) ride three tiny lane-packed add-AllReduces issued as soon as
their inputs finish, so their latency hides under the streaming passes.

Numerics: activations are round()ed to ints in [-127,127] and weights to
{-1,0,1} ({-2,0,2} for the sign-quantized W_v/W_d, with the 0.5 folded
into the dequant scales); all are exactly representable in fp16, and fp32
PSUM accumulation of <=2048 such products is exact, so the projection
matmuls are exact.  When the biases are zero (the benchmark instance),
projection outputs stay RAW integer sums and the dequantization scales
fold downstream (softmax exp scale, softmax reciprocal, LN input scale).
W_qkv is staged column-slab-tiled by the host so its f16 copy lives in
SBUF from the abs pass onward: it is quantized in place (magic-number
round at fp16 DVE rate) and consumed directly by the QK matmuls with no
weight reload DMAs.
"""

import math
import sys

import numpy as np

sys.path.insert(0, "/opt/trn_rl_repo")

import concourse.bacc as bacc
import concourse.bass as bass
import concourse.bass_isa as bass_isa
import concourse.mybir as mybir
import concourse.tile as tile

F32 = mybir.dt.float32
F16 = mybir.dt.float16
BF16 = mybir.dt.bfloat16
AF = mybir.ActivationFunctionType
OP = mybir.AluOpType

P = 128
H = 2048
S = 2048
B = 4
NH = 16
HD = 128
NCORES = 8
TOK = S                # tokens per batch
HB = H // P            # 16 h blocks
NHC = NH // 2          # 8 heads per core
HALF = TOK // 2        # 1024 tokens per core after reduce-scatter
MAGIC = float(2 ** 23)
M16 = 1024.0
INV_SQD = 1.0 / math.sqrt(HD)
LN_EPS = 1e-5
PAIRS = [[0, 1], [2, 3], [4, 5], [6, 7]]
ALL8 = [list(range(NCORES))]
# dense token-block order: chunk j covers my tokens 256j..256j+255 plus the
# partner's, so ReduceScatter chunk j can fire after 4 blocks
TBORD = [0, 1, 8, 9, 2, 3, 10, 11, 4, 5, 12, 13, 6, 7, 14, 15]


def _chunks(count, width, base=0):
    return [slice(base + i * width, base + (i + 1) * width) for i in range(count)]


def build_program(use_mask: bool, qk_bias_zero: bool, v_bias_zero: bool,
                  d_bias_zero: bool, ln_trivial: bool):
    nc = bacc.Bacc("TRN2", target_bir_lowering=False, debug=False,
                   enable_asserts=False, num_devices=NCORES)

    # ---- I/O --------------------------------------------------------------
    xt = nc.dram_tensor("xt", [H, TOK], F32, kind="ExternalInput")
    xr = nc.dram_tensor("xr", [HALF, H], F32, kind="ExternalInput")
    # W_qk^T column-slab-tiled: [ob, p, kb*128+c] = W^T[kb*128+p, ob*128+c]
    wqkt = nc.dram_tensor("wqkt", [16, P, 2048], F32, kind="ExternalInput")
    wvt = nc.dram_tensor("wvt", [H, 1024], F32, kind="ExternalInput")
    bqk = nc.dram_tensor("bqk", [P, 16], F32, kind="ExternalInput")
    bv = nc.dram_tensor("bv", [1, 1024], F32, kind="ExternalInput")
    wdt = nc.dram_tensor("wdt", [HALF, H], F32, kind="ExternalInput")
    bdh = nc.dram_tensor("bdh", [1, H], F32, kind="ExternalInput")
    maskt = nc.dram_tensor("maskt", [P, HB], F32, kind="ExternalInput")
    csel = nc.dram_tensor("csel", [1, 16], F32, kind="ExternalInput")
    lnw = nc.dram_tensor("lnw", [1, H], F32, kind="ExternalInput")
    lnb = nc.dram_tensor("lnb", [1, H], F32, kind="ExternalInput")
    out = nc.dram_tensor("out", [HALF, H], F32, kind="ExternalOutput")

    # ---- DRAM scratch ----------------------------------------------------
    wv_q = nc.dram_tensor("wv_q", [H, 1024], F16)
    wd_q = nc.dram_tensor("wd_q", [HALF, H], F16)
    qkt_d = nc.dram_tensor("qkt_d", [16, P, TOK], F16)
    vt_d = nc.dram_tensor("vt_d", [16, P, 1024], BF16)
    rs_in = nc.dram_tensor("rs_in", [2, 1024, H], BF16)
    rs_out = nc.dram_tensor("rs_out", [2, 512, H], BF16)
    c_add_i = nc.dram_tensor("c_add_i", [1, 16], F32)
    c_add_o = nc.dram_tensor("c_add_o", [1, 16], F32)
    c_mx_i = nc.dram_tensor("c_mx_i", [1, 16], F32)
    c_mx_o = nc.dram_tensor("c_mx_o", [1, 16], F32)
    c_wd_i = nc.dram_tensor("c_wd_i", [1, 16], F32)
    c_wd_o = nc.dram_tensor("c_wd_o", [1, 16], F32)
    c_mc_i = nc.dram_tensor("c_mc_i", [1, 16], F32)
    c_mc_o = nc.dram_tensor("c_mc_o", [1, 16], F32)

    with tile.TileContext(nc) as tc:
        _emit(tc, locals(), use_mask, qk_bias_zero, v_bias_zero,
              d_bias_zero, ln_trivial)

    nc.compile()
    return nc


def _emit(tc, T, use_mask, qk_bias_zero, v_bias_zero, d_bias_zero, ln_trivial):
    nc = tc.nc
    xt, xr, wqkt, wvt, bqk, bv, wdt, bdh = (T["xt"], T["xr"], T["wqkt"],
                                            T["wvt"], T["bqk"], T["bv"],
                                            T["wdt"], T["bdh"])
    maskt, lnw, lnb, out = T["maskt"], T["lnw"], T["lnb"], T["out"]
    csel = T["csel"]
    wv_q, wd_q, qkt_d = T["wv_q"], T["wd_q"], T["qkt_d"]
    vt_d = T["vt_d"]
    rs_in, rs_out = T["rs_in"], T["rs_out"]
    c_add_i, c_add_o = T["c_add_i"], T["c_add_o"]
    c_mx_i, c_mx_o = T["c_mx_i"], T["c_mx_o"]
    c_wd_i, c_wd_o = T["c_wd_i"], T["c_wd_o"]
    c_mc_i, c_mc_o = T["c_mc_i"], T["c_mc_o"]

    from contextlib import ExitStack

    est = ExitStack()
    with est:
        smalls = est.enter_context(tc.tile_pool(name="smalls", bufs=1))
        stream = est.enter_context(tc.tile_pool(name="stream", bufs=3))
        stream2 = est.enter_context(tc.tile_pool(name="stream2", bufs=2))
        red = est.enter_context(tc.tile_pool(name="red", bufs=4))

        def sc_tile(name, shape=(1, 1)):
            return smalls.tile(list(shape), F32, tag=name, name=name)

        def bcast(name, src):
            b = sc_tile(name, (P, 1))
            nc.gpsimd.partition_broadcast(b[:], src[:])
            return b

        ones_col = smalls.tile([P, 1], BF16, tag="ones_col")
        nc.vector.memset(ones_col[:], 1.0)
        ones_row = smalls.tile([1, P], BF16, tag="ones_row")
        nc.vector.memset(ones_row[:], 1.0)
        magic_b = smalls.tile([P, 1], F32, tag="magic_b")
        nc.vector.memset(magic_b[:], MAGIC)
        csb = smalls.tile([1, 16], F32, tag="csb")
        nc.sync.dma_start(csb[:], csel[:, :])
        bqk_sb = None
        if not qk_bias_zero:
            bqk_sb = smalls.tile([P, 16], F32, tag="bqk_sb")
            nc.sync.dma_start(bqk_sb[:], bqk[:, :])
        mask_sb = None
        if use_mask:
            mask_sb = smalls.tile([P, HB], F32, tag="mask_sb")
            nc.sync.dma_start(mask_sb[:], maskt[:, :])

        # ============ Stage 0a: max|x| pass -> AR_x (lanes 2..9) ===========
        xmax = sc_tile("xmax", (P, 1))
        for t in range(HB):
            xf = stream.tile([P, TOK], F32, tag="st32")
            nc.sync.dma_start(xf[:], xt[t * P:(t + 1) * P, :])
            r = red.tile([P, 1], F32, tag="xred")
            nc.vector.tensor_reduce(r[:], xf[:], axis=mybir.AxisListType.X,
                                    op=OP.max, apply_absolute_value=True)
            if t == 0:
                nc.vector.tensor_copy(xmax[:], r[:])
            else:
                nc.vector.tensor_tensor(xmax[:], xmax[:], r[:], OP.max)
        xmaxr = sc_tile("xmaxr", (P, 1))
        nc.gpsimd.partition_all_reduce(xmaxr[:], xmax[:], channels=P,
                                       reduce_op=bass_isa.ReduceOp.max)
        zpx = sc_tile("zpx", (1, 16))
        nc.vector.memset(zpx[:], 0.0)
        nc.vector.tensor_scalar(zpx[0:1, 2:10], csb[0:1, 0:8],
                                xmaxr[0:1, 0:1], None, OP.mult)
        nc.gpsimd.dma_start(c_mx_i[:, :], zpx[:])
        nc.gpsimd.collective_compute(
            "AllReduce", OP.add, replica_groups=ALL8,
            ins=[c_mx_i[:, :].opt()], outs=[c_mx_o[:, :].opt()])

        # ============ Stage 0b: |W| abs passes -> AR_A, AR_B ===============
        # W_qk slabs also cast to f16 and KEPT in SBUF for in-place quantize.
        accA = sc_tile("accA", (P, 1))
        accB = sc_tile("accB", (P, 1))


        s1es = ExitStack()
        xq_pool = s1es.enter_context(tc.tile_pool(name="xq", bufs=HB))
        wq_es = ExitStack()
        wq_pool = wq_es.enter_context(tc.tile_pool(name="wq16", bufs=HB))

        wq16 = []
        for ob in range(HB):
            wf = stream.tile([P, 2048], F32, tag="st32")
            nc.sync.dma_start(wf[:], wqkt[ob, :, :])
            r = red.tile([P, 1], F32, tag="wred")
            nc.vector.tensor_reduce(r[:], wf[:], axis=mybir.AxisListType.X,
                                    op=OP.add, apply_absolute_value=True)
            if ob == 0:
                nc.vector.tensor_copy(accA[:], r[:])
            else:
                nc.vector.tensor_tensor(accA[:], accA[:], r[:], OP.add)
            w16 = wq_pool.tile([P, 2048], F16, tag="wq16")
            nc.vector.tensor_copy(w16[:], wf[:])
            wq16.append(w16)

        for t in range(HB):
            wf = stream2.tile([P, 1024], F32, tag="sv32")
            nc.sync.dma_start(wf[:], wvt[t * P:(t + 1) * P, :])
            r = red.tile([P, 1], F32, tag="wred")
            nc.vector.tensor_reduce(r[:], wf[:], axis=mybir.AxisListType.X,
                                    op=OP.add, apply_absolute_value=True)
            nc.vector.tensor_tensor(accA[:], accA[:], r[:], OP.add)

        accAr = sc_tile("accAr", (P, 1))
        nc.gpsimd.partition_all_reduce(accAr[:], accA[:], channels=P,
                                       reduce_op=bass_isa.ReduceOp.add)
        zpa = sc_tile("zpa", (1, 16))
        nc.vector.memset(zpa[:], 0.0)
        nc.vector.tensor_copy(zpa[0:1, 0:1], accAr[0:1, 0:1])
        nc.gpsimd.dma_start(c_add_i[:, :], zpa[:])
        nc.gpsimd.collective_compute(
            "AllReduce", OP.add, replica_groups=ALL8,
            ins=[c_add_i[:, :].opt()], outs=[c_add_o[:, :].opt()])

        for t in range(HALF // P):
            wf = stream.tile([P, 2048], F32, tag="st32")
            nc.sync.dma_start(wf[:], wdt[t * P:(t + 1) * P, :])
            r = red.tile([P, 1], F32, tag="wred")
            nc.vector.tensor_reduce(r[:], wf[:], axis=mybir.AxisListType.X,
                                    op=OP.add, apply_absolute_value=True)
            if t == 0:
                nc.vector.tensor_copy(accB[:], r[:])
            else:
                nc.vector.tensor_tensor(accB[:], accB[:], r[:], OP.add)
        accBr = sc_tile("accBr", (P, 1))
        nc.gpsimd.partition_all_reduce(accBr[:], accB[:], channels=P,
                                       reduce_op=bass_isa.ReduceOp.add)
        zpb = sc_tile("zpb", (1, 16))
        nc.vector.memset(zpb[:], 0.0)
        nc.vector.tensor_copy(zpb[0:1, 1:2], accBr[0:1, 0:1])
        nc.gpsimd.dma_start(c_wd_i[:, :], zpb[:])
        nc.gpsimd.collective_compute(
            "AllReduce", OP.add, replica_groups=ALL8,
            ins=[c_wd_i[:, :].opt()], outs=[c_wd_o[:, :].opt()])

        # ============ scales (x first: xq is on the critical path) =========
        xm = sc_tile("xm", (1, 16))
        nc.sync.dma_start(xm[:], c_mx_o[:, :])
        xmx = sc_tile("xmx")
        nc.vector.tensor_reduce(xmx[:], xm[0:1, 2:10],
                                axis=mybir.AxisListType.X, op=OP.max)
        xm1 = sc_tile("xm1")
        nc.vector.tensor_scalar(xm1[:], xmx[:], 1e-8, None, OP.add)
        rxm = sc_tile("rxm")
        nc.vector.reciprocal(rxm[:], xm1[:])
        sx = sc_tile("sx")
        nc.vector.tensor_scalar(sx[:], rxm[:], 127.0, None, OP.mult)
        sx_b = bcast("sx_b", sx)

        # quantize x: round(x*sx) -> f16 ints, kept in SBUF
        xq = []
        for kb in range(HB):
            xf = stream.tile([P, TOK], F32, tag="st32")
            nc.sync.dma_start(xf[:], xt[kb * P:(kb + 1) * P, :])
            t1 = stream.tile([P, TOK], F32, tag="st32")
            nc.scalar.activation(t1[:], xf[:], AF.Identity, bias=magic_b[:],
                                 scale=sx_b[:])
            q = xq_pool.tile([P, TOK], F16, tag="xq")
            nc.vector.tensor_scalar(q[:], t1[:], MAGIC, None, OP.subtract)
            xq.append(q)

        # gamma_qkv = sum|W_qkv|/(3H*H)+1e-5 (all-8 add = 4x full sum)
        wsA = sc_tile("wsA", (1, 16))
        nc.sync.dma_start(wsA[:], c_add_o[:, :])
        gq = sc_tile("gq")
        nc.vector.tensor_scalar(gq[:], wsA[0:1, 0:1],
                                1.0 / (4 * 3 * H * H), 1e-5, OP.mult, OP.add)
        igq = sc_tile("igq")
        nc.vector.reciprocal(igq[:], gq[:])
        igq_b = bcast("igq_b", igq)

        # quantize W_qk slabs IN PLACE (f16 magic round, clip to [-1,1])
        for ob in range(HB):
            t1 = stream2.tile([P, 2048], F16, tag="q16")
            nc.vector.tensor_scalar(t1[:], wq16[ob][:], igq_b[:], M16,
                                    OP.mult, OP.add)
            t2 = stream2.tile([P, 2048], F16, tag="q16")
            nc.vector.tensor_scalar(t2[:], t1[:], M16, 1.0,
                                    OP.subtract, OP.min)
            nc.vector.tensor_scalar(wq16[ob][:], t2[:], -1.0, None, OP.max)

        # remaining scales
        al_t = sc_tile("al_t")
        nc.vector.tensor_tensor(al_t[:], gq[:], xm1[:], OP.mult)
        alpha = sc_tile("alpha")
        nc.vector.tensor_scalar(alpha[:], al_t[:], 1.0 / 127.0, None, OP.mult)
        alpha_b = bcast("alpha_b", alpha)
        a2 = sc_tile("a2")
        nc.vector.tensor_tensor(a2[:], alpha[:], alpha[:], OP.mult)
        nc.vector.tensor_scalar(a2[:], a2[:], INV_SQD, None, OP.mult)
        a2_b = bcast("a2_b", a2)
        # sign-route (W_v, W_d) gives {-2,0,2}; alpv/alphad carry the 0.5
        alpv = sc_tile("alpv")
        nc.vector.tensor_scalar(alpv[:], alpha[:], 0.5, None, OP.mult)
        alpv_b = bcast("alpv_b", alpv)
        ntq = sc_tile("ntq")
        nc.vector.tensor_scalar(ntq[:], gq[:], -0.5, None, OP.mult)
        ntq_b = bcast("ntq_b", ntq)
        ptq = sc_tile("ptq")
        nc.vector.tensor_scalar(ptq[:], gq[:], 0.5, None, OP.mult)
        ptq_b = bcast("ptq_b", ptq)

        wsB = sc_tile("wsB", (1, 16))
        nc.sync.dma_start(wsB[:], c_wd_o[:, :])
        gd = sc_tile("gd")
        nc.vector.tensor_scalar(gd[:], wsB[0:1, 1:2],
                                1.0 / (4 * H * H), 1e-5, OP.mult, OP.add)
        ntd = sc_tile("ntd")
        nc.vector.tensor_scalar(ntd[:], gd[:], -0.5, None, OP.mult)
        ntd_b = bcast("ntd_b", ntd)
        ptd = sc_tile("ptd")
        nc.vector.tensor_scalar(ptd[:], gd[:], 0.5, None, OP.mult)
        ptd_b = bcast("ptd_b", ptd)

        bvb = None
        if not v_bias_zero:
            bv_sb = smalls.tile([1, 1024], F32, tag="bv_sb")
            nc.sync.dma_start(bv_sb[:], bv[:, :])
            bvb = smalls.tile([P, 1024], F32, tag="bvb")
            nc.gpsimd.partition_broadcast(bvb[:], bv_sb[:])

        # sign-quantize W_v / W_d on the Scalar engine -> DRAM f16
        def quantize_w_sign(dram_in, dram_out, nrows, width, ntb, ptb):
            for t in range(nrows // P):
                wf = stream2.tile([P, width], F32, tag="sv32")
                nc.sync.dma_start(wf[:], dram_in[t * P:(t + 1) * P, :])
                s1 = stream.tile([P, width], F16, tag="sg16")
                nc.scalar.activation(s1[:], wf[:], AF.Sign, bias=ntb[:])
                s2 = stream.tile([P, width], F16, tag="sg16")
                nc.scalar.activation(s2[:], wf[:], AF.Sign, bias=ptb[:])
                t3 = stream.tile([P, width], F16, tag="sg16")
                nc.vector.tensor_tensor(t3[:], s1[:], s2[:], OP.add)
                nc.sync.dma_start(dram_out[t * P:(t + 1) * P, :], t3[:])

        quantize_w_sign(wvt, wv_q, H, 1024, ntq_b, ptq_b)
        quantize_w_sign(wdt, wd_q, HALF, H, ntd_b, ptd_b)

        # ============ Stage 1: QKV projection ==============================
        with tc.tile_pool(name="s1ev", bufs=3) as ev_pool, \
             tc.tile_pool(name="ps1", bufs=2, space="PSUM") as ps1:
            # Q^T and K^T: weights already in SBUF (wq16 slabs); raw integer
            # sums when bias is zero (alpha^2 folded into exp scale).
            for ob in range(16):
                psum = ps1.tile([P, TOK], F32, tag="ps")
                for kb in range(HB):
                    for sl in _chunks(4, 512):
                        nc.tensor.matmul(psum[:, sl],
                                         lhsT=wq16[ob][:, kb * P:(kb + 1) * P],
                                         rhs=xq[kb][:, sl],
                                         start=(kb == 0), stop=(kb == HB - 1))
                ev = ev_pool.tile([P, TOK], F16, tag="ev")
                if qk_bias_zero:
                    nc.vector.tensor_copy(ev[:], psum[:])
                else:
                    nc.scalar.activation(ev[:], psum[:], AF.Identity,
                                         bias=bqk_sb[:, ob:ob + 1],
                                         scale=alpha_b[:])
                nc.sync.dma_start(qkt_d[ob, :, :], ev[:])
        wq_es.close()

        with tc.tile_pool(name="wv_sb", bufs=HB) as wv_pool, \
             tc.tile_pool(name="evv", bufs=3) as evv_pool, \
             tc.tile_pool(name="ps1v", bufs=2, space="PSUM") as ps1v:
            # V: [tok, 1024] per token block, kept in SBUF bf16 (raw ints
            # when bias zero; alpv folded into the softmax reciprocal).
            wv_list = []
            for kb in range(HB):
                wvq = wv_pool.tile([P, 1024], F16, tag="wv_sb")
                nc.sync.dma_start(wvq[:], wv_q[kb * P:(kb + 1) * P, :])
                wv_list.append(wvq)
            for tb in range(HB):
                psum = ps1v.tile([P, 1024], F32, tag="ps")
                for kb in range(HB):
                    for sl in _chunks(2, 512):
                        nc.tensor.matmul(
                            psum[:, sl],
                            lhsT=xq[kb][:, tb * P:(tb + 1) * P],
                            rhs=wv_list[kb][:, sl],
                            start=(kb == 0), stop=(kb == HB - 1))
                v = evv_pool.tile([P, 1024], BF16, tag="vt")
                if v_bias_zero:
                    nc.vector.tensor_copy(v[:], psum[:])
                else:
                    nc.vector.scalar_tensor_tensor(v[:], psum[:], alpv_b[:],
                                                   bvb[:], OP.mult, OP.add)
                nc.sync.dma_start(vt_d[tb, :, :], v[:])
        s1es.close()

        # ============ Stage 2: attention ===================================
        # The psc eviction is split: a cheap TS (raw ctx * alpv) frees the
        # ctx PSUM immediately; the slow normalize chain (broadcast 1/denom,
        # DVE reciprocal ~6.5us, multiply) is deferred into the NEXT half's
        # emission so it hides under that half's matmuls.  Normalized ctx is
        # kept in SBUF as f16 for stage 3.
        mxacc = sc_tile("mxacc", (P, 1))
        cn_es = ExitStack()
        cn_pool = cn_es.enter_context(tc.tile_pool(name="cn", bufs=16))
        cn_keep = {}
        state = {"first_mx": True, "pend": None}

        def finish_half(p):
            hh, qq, cnr, psd_s = p
            rb = rb_pool.tile([P, 1024], F32, tag="rb")
            nc.gpsimd.partition_broadcast(rb[:], psd_s[:])
            rbr = rb_pool.tile([P, 1024], F32, tag="rb")
            nc.vector.reciprocal(rbr[:], rb[:])
            cnf = cn_pool.tile([P, 1024], F16, tag="cnh")
            nc.vector.tensor_tensor(cnf[:], cnr[:], rbr[:], OP.mult)
            r = red.tile([P, 1], F32, tag="cmax")
            nc.vector.tensor_reduce(r[:], cnf[:], axis=mybir.AxisListType.X,
                                    op=OP.max, apply_absolute_value=True)
            if state["first_mx"]:
                nc.vector.tensor_copy(mxacc[:], r[:])
                state["first_mx"] = False
            else:
                nc.vector.tensor_tensor(mxacc[:], mxacc[:], r[:], OP.max)
            cn_keep[(hh, qq)] = cnf

        with tc.tile_pool(name="qkt", bufs=4) as qk_pool, \
             tc.tile_pool(name="vh", bufs=28) as vh_pool, \
             tc.tile_pool(name="et", bufs=20) as et_pool, \
             tc.tile_pool(name="rb", bufs=2) as rb_pool, \
             tc.tile_pool(name="cnr", bufs=2) as cnr_pool, \
             tc.tile_pool(name="rd", bufs=2) as rd_pool, \
             tc.tile_pool(name="ps2s", bufs=2, space="PSUM") as ps2s, \
             tc.tile_pool(name="ps2c", bufs=1, space="PSUM") as ps2c, \
             tc.tile_pool(name="ps2d", bufs=1, space="PSUM") as ps2d:
            for h in range(NHC):
                qt = qk_pool.tile([P, TOK], F16, tag="qt")
                nc.sync.dma_start(qt[:], qkt_d[h, :, :])
                kt = qk_pool.tile([P, TOK], F16, tag="kt")
                nc.sync.dma_start(kt[:], qkt_d[NHC + h, :, :])
                vh = []
                for kb in range(HB):
                    vk = vh_pool.tile([P, P], BF16, tag="vh")
                    nc.sync.dma_start(vk[:], vt_d[kb, :, h * P:(h + 1) * P])
                    vh.append(vk)

                for qh in range(2):
                    q0 = qh * 1024
                    et = []
                    for kb in range(HB):
                        pss = ps2s.tile([P, 1024], F32, tag="pss")
                        for sl, psl in zip(_chunks(2, 512, q0),
                                           _chunks(2, 512)):
                            nc.tensor.matmul(pss[:, psl],
                                             lhsT=kt[:, kb * P:(kb + 1) * P],
                                             rhs=qt[:, sl],
                                             start=True, stop=True)
                        e = et_pool.tile([P, 1024], BF16, tag="et")
                        nc.scalar.activation(
                            e[:], pss[:], AF.Exp,
                            bias=(mask_sb[:, kb:kb + 1] if use_mask else 0.0),
                            scale=(a2_b[:] if qk_bias_zero else INV_SQD))
                        et.append(e)

                    if state["pend"] is not None:
                        finish_half(state["pend"])
                        state["pend"] = None

                    psc = ps2c.tile([P, 1024], F32, tag="psc")
                    psd = ps2d.tile([1, 1024], F32, tag="psd")
                    for kb in range(HB):
                        vv = vh[kb][:]
                        for sl in _chunks(2, 512):
                            nc.tensor.matmul(psc[:, sl], lhsT=vv,
                                             rhs=et[kb][:, sl],
                                             start=(kb == 0),
                                             stop=(kb == HB - 1))
                        for sl in _chunks(2, 512):
                            nc.tensor.matmul(psd[:, sl], lhsT=ones_col[:],
                                             rhs=et[kb][:, sl],
                                             start=(kb == 0),
                                             stop=(kb == HB - 1))

                    cnr = cnr_pool.tile([P, 1024], F32, tag="cnr")
                    if v_bias_zero:
                        nc.vector.tensor_scalar(cnr[:], psc[:], alpv_b[:],
                                                None, OP.mult)
                    else:
                        nc.vector.tensor_copy(cnr[:], psc[:])
                    psd_s = rd_pool.tile([1, 1024], F32, tag="rd")
                    nc.vector.tensor_copy(psd_s[:], psd[:, :])
                    state["pend"] = (h, qh, cnr, psd_s)
            finish_half(state["pend"])
            state["pend"] = None


        # ============ ctx max AllReduce + quantize scales ==================
        mxr = sc_tile("mxr", (P, 1))
        nc.gpsimd.partition_all_reduce(mxr[:], mxacc[:], channels=P,
                                       reduce_op=bass_isa.ReduceOp.max)
        zpad3 = sc_tile("zpad3", (1, 16))
        nc.vector.memset(zpad3[:], 0.0)
        nc.vector.tensor_copy(zpad3[0:1, 0:1], mxr[0:1, 0:1])
        nc.gpsimd.dma_start(c_mc_i[:, :], zpad3[:])
        nc.gpsimd.collective_compute(
            "AllReduce", OP.max, replica_groups=ALL8,
            ins=[c_mc_i[:, :].opt()], outs=[c_mc_o[:, :].opt()])
        cm = sc_tile("cm", (1, 16))
        nc.sync.dma_start(cm[:], c_mc_o[:, :])

        cm1 = sc_tile("cm1")
        nc.vector.tensor_scalar(cm1[:], cm[0:1, 0:1], 1e-8, None, OP.add)
        rcm = sc_tile("rcm")
        nc.vector.reciprocal(rcm[:], cm1[:])
        sctx = sc_tile("sctx")
        nc.vector.tensor_scalar(sctx[:], rcm[:], 127.0, None, OP.mult)
        ad_t = sc_tile("ad_t")
        nc.vector.tensor_tensor(ad_t[:], gd[:], cm1[:], OP.mult)
        # extra 0.5: W_d was sign-quantized to {-2,0,2}
        alphad = sc_tile("alphad")
        nc.vector.tensor_scalar(alphad[:], ad_t[:], 0.5 / 127.0, None, OP.mult)
        sctx_b = bcast("sctx_b", sctx)
        alphad_b = bcast("alphad_b", alphad)

        bdb = None
        if not d_bias_zero:
            bd_sb = smalls.tile([1, H], F32, tag="bd_sb")
            nc.sync.dma_start(bd_sb[:], bdh[:, :])
            bdb = smalls.tile([P, H], F32, tag="bdb")
            nc.gpsimd.partition_broadcast(bdb[:], bd_sb[:])

        # ============ Stage 3: quantize ctx, dense, chunked RS =============
        with tc.tile_pool(name="cq", bufs=NHC) as cq_pool, \
             tc.tile_pool(name="wd_sb", bufs=NHC) as wd_pool, \
             tc.tile_pool(name="s3ev", bufs=3) as ev3_pool, \
             tc.tile_pool(name="ps3", bufs=2, space="PSUM") as ps3:
            wd_sb = []
            for kb in range(NHC):
                w = wd_pool.tile([P, H], F16, tag="wd_sb")
                nc.sync.dma_start(w[:], wd_q[kb * P:(kb + 1) * P, :])
                wd_sb.append(w)

            ctxq = []
            for h in range(NHC):
                q = cq_pool.tile([P, TOK], F16, tag="cq")
                for qh in range(2):
                    q0 = qh * 1024
                    t1 = stream2.tile([P, 1024], F16, tag="cq16")
                    nc.vector.tensor_scalar(t1[:], cn_keep[(h, qh)][:],
                                            sctx_b[:], M16, OP.mult, OP.add)
                    nc.vector.tensor_scalar(q[:, q0:q0 + 1024], t1[:], M16,
                                            None, OP.subtract)
                ctxq.append(q)

            # token-interleaved chunks: RS chunk j fires after its 4 token
            # blocks, overlapping the remaining dense matmuls; LN follows
            # per chunk.
            for j in range(2):
                for tb in TBORD[8 * j:8 * j + 8]:
                    psum = ps3.tile([P, TOK], F32, tag="ps")
                    for kb in range(NHC):
                        for sl in _chunks(4, 512):
                            nc.tensor.matmul(
                                psum[:, sl],
                                lhsT=ctxq[kb][:, tb * P:(tb + 1) * P],
                                rhs=wd_sb[kb][:, sl],
                                start=(kb == 0), stop=(kb == NHC - 1))
                    ev = ev3_pool.tile([P, TOK], BF16, tag="ev3")
                    if d_bias_zero:
                        # raw int sums; alphad folded into LN input scale
                        nc.vector.tensor_copy(ev[:], psum[:])
                    else:
                        nc.vector.scalar_tensor_tensor(
                            ev[:], psum[:], alphad_b[:], bdb[:],
                            OP.mult, OP.add)
                    off = (0 if tb < 8 else 512) + (tb % 4) * P
                    nc.sync.dma_start(rs_in[j, off:off + P, :], ev[:])
                nc.gpsimd.collective_compute(
                    "ReduceScatter", OP.add, replica_groups=PAIRS,
                    ins=[rs_in[j, :, :].opt()], outs=[rs_out[j, :, :].opt()])
        cn_es.close()

        lnwb = lnbb = None
        if not ln_trivial:
            lnw_sb = smalls.tile([1, H], F32, tag="lnw_sb")
            nc.sync.dma_start(lnw_sb[:], lnw[:, :])
            lnwb = smalls.tile([P, H], F32, tag="lnwb")
            nc.gpsimd.partition_broadcast(lnwb[:], lnw_sb[:])
            lnb_sb = smalls.tile([1, H], F32, tag="lnb_sb")
            nc.sync.dma_start(lnb_sb[:], lnb[:, :])
            lnbb = smalls.tile([P, H], F32, tag="lnbb")
            nc.gpsimd.partition_broadcast(lnbb[:], lnb_sb[:])

        # ============ Stage 4: residual + layernorm ========================
        with tc.tile_pool(name="ln", bufs=2) as ln_pool, \
             tc.tile_pool(name="lns", bufs=4) as lns_pool:
            for m in range(HALF // P):
                r_t = ln_pool.tile([P, H], BF16, tag="lnr")
                nc.sync.dma_start(r_t[:],
                                  rs_out[m // 4, (m % 4) * P:(m % 4) * P + P, :])
                x_t = ln_pool.tile([P, H], F32, tag="lnx")
                nc.sync.dma_start(x_t[:], xr[m * P:(m + 1) * P, :])

                y = ln_pool.tile([P, H], F32, tag="lny")
                ysum = lns_pool.tile([P, 1], F32, tag="ysum")
                dscale = alphad_b[:] if d_bias_zero else 1.0
                nc.vector.scalar_tensor_tensor(y[:], r_t[:], dscale, x_t[:],
                                               OP.mult, OP.add,
                                               accum_out=ysum[:])
                mu = lns_pool.tile([P, 1], F32, tag="mu")
                nc.vector.tensor_scalar(mu[:], ysum[:], 1.0 / H, None, OP.mult)
                nmu = lns_pool.tile([P, 1], F32, tag="nmu")
                nc.vector.tensor_scalar(nmu[:], mu[:], -1.0, None, OP.mult)

                sq = ln_pool.tile([P, H], F32, tag="lnsq")
                sqs = lns_pool.tile([P, 1], F32, tag="sqs")
                nc.scalar.activation(sq[:], y[:], AF.Square,
                                     bias=nmu[:], scale=1.0,
                                     accum_out=sqs[:])
                v1 = lns_pool.tile([P, 1], F32, tag="v1")
                nc.vector.tensor_scalar(v1[:], sqs[:], 1.0 / H, LN_EPS,
                                        OP.mult, OP.add)
                v2 = lns_pool.tile([P, 1], F32, tag="v2")
                nc.vector.reciprocal(v2[:], v1[:])
                rstd = lns_pool.tile([P, 1], F32, tag="rstd")
                nc.scalar.activation(rstd[:], v2[:], AF.Sqrt)
                nmr = lns_pool.tile([P, 1], F32, tag="nmr")
                nc.vector.tensor_tensor(nmr[:], nmu[:], rstd[:], OP.mult)

                yn = ln_pool.tile([P, H], F32, tag="lnyn")
                nc.scalar.activation(yn[:], y[:], AF.Identity,
                                     bias=nmr[:], scale=rstd[:])
                if not ln_trivial:
                    nc.vector.tensor_tensor(yn[:], yn[:], lnwb[:], OP.mult)
                    nc.vector.tensor_tensor(yn[:], yn[:], lnbb[:], OP.add)
                nc.sync.dma_start(out[m * P:(m + 1) * P, :], yn[:])


# ======================= host side =======================================

def make_in_maps(hidden_states, attention_mask, W_qkv, b_qkv, W_dense,
                 b_dense, ln_w, ln_b):
    x = np.asarray(hidden_states, dtype=np.float32)
    mask = np.asarray(attention_mask, dtype=np.float32)
    Wq = np.asarray(W_qkv, dtype=np.float32)
    bq = np.asarray(b_qkv, dtype=np.float32)
    Wd = np.asarray(W_dense, dtype=np.float32)
    bd = np.asarray(b_dense, dtype=np.float32)
    lw = np.asarray(ln_w, dtype=np.float32)
    lb = np.asarray(ln_b, dtype=np.float32)

    in_maps = []
    for c in range(NCORES):
        b, g = c // 2, c % 2
        sl = slice(g * 1024, (g + 1) * 1024)
        wq_g = Wq[sl, :]
        wk_g = Wq[2048 + g * 1024:2048 + (g + 1) * 1024, :]
        wv_g = Wq[4096 + g * 1024:4096 + (g + 1) * 1024, :]
        bq_g = bq[sl]
        bk_g = bq[2048 + g * 1024:2048 + (g + 1) * 1024]
        bv_g = bq[4096 + g * 1024:4096 + (g + 1) * 1024]
        W2 = np.concatenate([wq_g, wk_g], axis=0).T  # [h, out]
        wq_tiled = np.ascontiguousarray(
            W2.reshape(16, P, 16, P).transpose(2, 1, 0, 3).reshape(16, P, H))
        in_maps.append({
            "xt": np.ascontiguousarray(x[b].T),
            "xr": np.ascontiguousarray(x[b, g * 1024:(g + 1) * 1024, :]),
            "wqkt": wq_tiled,
            "wvt": np.ascontiguousarray(wv_g.T),
            "bqk": np.ascontiguousarray(
                np.concatenate([bq_g, bk_g]).reshape(16, P).T),
            "bv": bv_g.reshape(1, 1024).copy(),
            "wdt": np.ascontiguousarray(Wd[:, g * 1024:(g + 1) * 1024].T),
            "bdh": (bd * 0.5).reshape(1, H).copy(),
            "maskt": np.ascontiguousarray(mask[b, 0, 0, :].reshape(HB, P).T),
            "csel": np.eye(16, dtype=np.float32)[2 + c].reshape(1, 16).copy(),
            "lnw": lw.reshape(1, H).copy(),
            "lnb": lb.reshape(1, H).copy(),
        })
    return in_maps


def build_flags(attention_mask, b_qkv, b_dense, ln_w, ln_b):
    return (
        bool(np.any(np.asarray(attention_mask) != 0.0)),
        bool(np.all(np.asarray(b_qkv)[:4096] == 0.0)),
        bool(np.all(np.asarray(b_qkv)[4096:] == 0.0)),
        bool(np.all(np.asarray(b_dense) == 0.0)),
        bool(np.all(np.asarray(ln_w) == 1.0) and np.all(np.asarray(ln_b) == 0.0)),
    )


def assemble_output(results):
    full = np.empty((B, S, H), dtype=np.float32)
    for c in range(NCORES):
        b, g = c // 2, c % 2
        full[b, g * 1024:(g + 1) * 1024, :] = results[c]["out"]
    return full


_CACHE = {}


def _get_program(flags):
    if flags not in _CACHE:
        _CACHE[flags] = build_program(*flags)
    return _CACHE[flags]


def _ensure_ntff_hook():
    """Provide antenv.axon_hooks (missing in this image) so trace=True can
    capture NTFF profiles through the axon PJRT plugin."""
    import types

    try:
        import antenv.axon_hooks  # noqa: F401
        return
    except ImportError:
        pass
    try:
        import antenv
    except ImportError:
        return
    mod = types.ModuleType("antenv.axon_hooks")
    holder = {"h": None}
    mod.set_axon_ntff_profile_hook = lambda h: holder.__setitem__("h", h)
    mod.get_axon_ntff_profile_hook = lambda: holder["h"]
    sys.modules["antenv.axon_hooks"] = mod
    antenv.axon_hooks = mod
    try:
        if "/root/.axon_site" not in sys.path:
            sys.path.insert(0, "/root/.axon_site")
        from trn_agent_boot.trn_boot import _ntff_profile_via_ctypes
        h = _ntff_profile_via_ctypes("/opt/axon/libaxon_pjrt.so")
        if h is not None:
            mod.set_axon_ntff_profile_hook(h)
    except Exception:
        pass


def kernel(hidden_states, attention_mask, W_qkv, b_qkv, W_dense, b_dense,
           ln_w, ln_b, trace=False):
    from concourse.bass_utils import run_bass_kernel_spmd

    flags = build_flags(attention_mask, b_qkv, b_dense, ln_w, ln_b)
    nc = _get_program(flags)
    in_maps = make_in_maps(hidden_states, attention_mask, W_qkv, b_qkv,
                           W_dense, b_dense, ln_w, ln_b)
    if trace:
        _ensure_ntff_hook()
        try:
            res = run_bass_kernel_spmd(nc, in_maps,
                                       core_ids=list(range(NCORES)),
                                       trace=True)
        except Exception as e:
            print("trace run failed (%s); retrying untraced" % e)
            res = run_bass_kernel_spmd(nc, in_maps,
                                       core_ids=list(range(NCORES)),
                                       trace=False)
    else:
        res = run_bass_kernel_spmd(nc, in_maps, core_ids=list(range(NCORES)),
                                   trace=False)
    out = assemble_output(res.results)
    kernel.last_result = res
    return out


# revision 27
# speedup vs baseline: 1.1138x; 1.1138x over previous
"""BitNet attention layer (quantized QKV + attention + quantized dense + LN)
as a Bass/Tile SPMD kernel for 8 Trainium2 NeuronCores.

Sharding: core c = 2*b + g handles batch b (of 4) and head-group g (of 2,
8 heads each).  QKV projection + attention are fully local per core
(tensor-parallel over heads, data-parallel over batch); the dense output
projection is tensor-parallel over its input dim, pair-reduced with a
token-interleaved chunked ReduceScatter that pipelines with the dense
matmuls, so each core finishes residual+layernorm on its half of the
batch's tokens.  Cross-core scalars (weight abs-means, activation
abs-maxes) ride three tiny lane-packed add-AllReduces issued as soon as
their inputs finish, so their latency hides under the streaming passes.

Numerics: activations are round()ed to ints in [-127,127] and weights to
{-1,0,1} ({-2,0,2} for the sign-quantized W_v/W_d, with the 0.5 folded
into the dequant scales); all are exactly representable in fp16, and fp32
PSUM accumulation of <=2048 such products is exact, so the projection
matmuls are exact.  When the biases are zero (the benchmark instance),
projection outputs stay RAW integer sums and the dequantization scales
fold downstream (softmax exp scale, softmax reciprocal, LN input scale).
W_qkv is staged column-slab-tiled by the host so its f16 copy lives in
SBUF from the abs pass onward: it is quantized in place (magic-number
round at fp16 DVE rate) and consumed directly by the QK matmuls with no
weight reload DMAs.
"""

import math
import sys

import numpy as np

sys.path.insert(0, "/opt/trn_rl_repo")

import concourse.bacc as bacc
import concourse.bass as bass
import concourse.bass_isa as bass_isa
import concourse.mybir as mybir
import concourse.tile as tile

F32 = mybir.dt.float32
F16 = mybir.dt.float16
BF16 = mybir.dt.bfloat16
AF = mybir.ActivationFunctionType
OP = mybir.AluOpType

P = 128
H = 2048
S = 2048
B = 4
NH = 16
HD = 128
NCORES = 8
TOK = S                # tokens per batch
HB = H // P            # 16 h blocks
NHC = NH // 2          # 8 heads per core
HALF = TOK // 2        # 1024 tokens per core after reduce-scatter
MAGIC = float(2 ** 23)
M16 = 1024.0
INV_SQD = 1.0 / math.sqrt(HD)
LN_EPS = 1e-5
PAIRS = [[0, 1], [2, 3], [4, 5], [6, 7]]
ALL8 = [list(range(NCORES))]
# dense token-block order: chunk j covers my tokens 256j..256j+255 plus the
# partner's, so ReduceScatter chunk j can fire after 4 blocks
TBORD = [0, 1, 8, 9, 2, 3, 10, 11, 4, 5, 12, 13, 6, 7, 14, 15]


def _chunks(count, width, base=0):
    return [slice(base + i * width, base + (i + 1) * width) for i in range(count)]


def build_program(use_mask: bool, qk_bias_zero: bool, v_bias_zero: bool,
                  d_bias_zero: bool, ln_trivial: bool):
    nc = bacc.Bacc("TRN2", target_bir_lowering=False, debug=False,
                   enable_asserts=False, num_devices=NCORES)

    # ---- I/O --------------------------------------------------------------
    xt = nc.dram_tensor("xt", [H, TOK], F32, kind="ExternalInput")
    xr = nc.dram_tensor("xr", [HALF, H], F32, kind="ExternalInput")
    # W_qk^T column-slab-tiled: [ob, p, kb*128+c] = W^T[kb*128+p, ob*128+c]
    wqkt = nc.dram_tensor("wqkt", [16, P, 2048], F32, kind="ExternalInput")
    wvt = nc.dram_tensor("wvt", [H, 1024], F32, kind="ExternalInput")
    bqk = nc.dram_tensor("bqk", [P, 16], F32, kind="ExternalInput")
    bv = nc.dram_tensor("bv", [1, 1024], F32, kind="ExternalInput")
    wdt = nc.dram_tensor("wdt", [HALF, H], F32, kind="ExternalInput")
    bdh = nc.dram_tensor("bdh", [1, H], F32, kind="ExternalInput")
    maskt = nc.dram_tensor("maskt", [P, HB], F32, kind="ExternalInput")
    csel = nc.dram_tensor("csel", [1, 16], F32, kind="ExternalInput")
    lnw = nc.dram_tensor("lnw", [1, H], F32, kind="ExternalInput")
    lnb = nc.dram_tensor("lnb", [1, H], F32, kind="ExternalInput")
    out = nc.dram_tensor("out", [HALF, H], F32, kind="ExternalOutput")

    # ---- DRAM scratch ----------------------------------------------------
    wv_q = nc.dram_tensor("wv_q", [H, 1024], F16)
    wd_q = nc.dram_tensor("wd_q", [HALF, H], F16)
    qkt_d = nc.dram_tensor("qkt_d", [16, P, TOK], F16)
    vt_d = nc.dram_tensor("vt_d", [16, P, 1024], BF16)
    rs_in = nc.dram_tensor("rs_in", [4, 512, H], BF16)
    rs_out = nc.dram_tensor("rs_out", [4, 256, H], BF16)
    c_add_i = nc.dram_tensor("c_add_i", [1, 16], F32)
    c_add_o = nc.dram_tensor("c_add_o", [1, 16], F32)
    c_mx_i = nc.dram_tensor("c_mx_i", [1, 16], F32)
    c_mx_o = nc.dram_tensor("c_mx_o", [1, 16], F32)
    c_wd_i = nc.dram_tensor("c_wd_i", [1, 16], F32)
    c_wd_o = nc.dram_tensor("c_wd_o", [1, 16], F32)
    c_mc_i = nc.dram_tensor("c_mc_i", [1, 16], F32)
    c_mc_o = nc.dram_tensor("c_mc_o", [1, 16], F32)

    with tile.TileContext(nc) as tc:
        _emit(tc, locals(), use_mask, qk_bias_zero, v_bias_zero,
              d_bias_zero, ln_trivial)

    nc.compile()
    return nc


def _emit(tc, T, use_mask, qk_bias_zero, v_bias_zero, d_bias_zero, ln_trivial):
    nc = tc.nc
    xt, xr, wqkt, wvt, bqk, bv, wdt, bdh = (T["xt"], T["xr"], T["wqkt"],
                                            T["wvt"], T["bqk"], T["bv"],
                                            T["wdt"], T["bdh"])
    maskt, lnw, lnb, out = T["maskt"], T["lnw"], T["lnb"], T["out"]
    csel = T["csel"]
    wv_q, wd_q, qkt_d = T["wv_q"], T["wd_q"], T["qkt_d"]
    vt_d = T["vt_d"]
    rs_in, rs_out = T["rs_in"], T["rs_out"]
    c_add_i, c_add_o = T["c_add_i"], T["c_add_o"]
    c_mx_i, c_mx_o = T["c_mx_i"], T["c_mx_o"]
    c_wd_i, c_wd_o = T["c_wd_i"], T["c_wd_o"]
    c_mc_i, c_mc_o = T["c_mc_i"], T["c_mc_o"]

    from contextlib import ExitStack

    est = ExitStack()
    with est:
        smalls = est.enter_context(tc.tile_pool(name="smalls", bufs=1))
        stream = est.enter_context(tc.tile_pool(name="stream", bufs=3))
        stream2 = est.enter_context(tc.tile_pool(name="stream2", bufs=2))
        red = est.enter_context(tc.tile_pool(name="red", bufs=4))

        def sc_tile(name, shape=(1, 1)):
            return smalls.tile(list(shape), F32, tag=name, name=name)

        def bcast(name, src):
            b = sc_tile(name, (P, 1))
            nc.gpsimd.partition_broadcast(b[:], src[:])
            return b

        ones_col = smalls.tile([P, 1], BF16, tag="ones_col")
        nc.vector.memset(ones_col[:], 1.0)
        ones_row = smalls.tile([1, P], BF16, tag="ones_row")
        nc.vector.memset(ones_row[:], 1.0)
        csb = smalls.tile([1, 16], F32, tag="csb")
        nc.sync.dma_start(csb[:], csel[:, :])
        bqk_sb = None
        if not qk_bias_zero:
            bqk_sb = smalls.tile([P, 16], F32, tag="bqk_sb")
            nc.sync.dma_start(bqk_sb[:], bqk[:, :])
        mask_sb = None
        if use_mask:
            mask_sb = smalls.tile([P, HB], F32, tag="mask_sb")
            nc.sync.dma_start(mask_sb[:], maskt[:, :])

        # ============ Stage 0a: max|x| pass -> AR_x (lanes 2..9) ===========
        xmax = sc_tile("xmax", (P, 1))
        for t in range(HB):
            xf = stream.tile([P, TOK], F32, tag="st32")
            nc.sync.dma_start(xf[:], xt[t * P:(t + 1) * P, :])
            r = red.tile([P, 1], F32, tag="xred")
            nc.vector.tensor_reduce(r[:], xf[:], axis=mybir.AxisListType.X,
                                    op=OP.max, apply_absolute_value=True)
            if t == 0:
                nc.vector.tensor_copy(xmax[:], r[:])
            else:
                nc.vector.tensor_tensor(xmax[:], xmax[:], r[:], OP.max)
        xmaxr = sc_tile("xmaxr", (P, 1))
        nc.gpsimd.partition_all_reduce(xmaxr[:], xmax[:], channels=P,
                                       reduce_op=bass_isa.ReduceOp.max)
        zpx = sc_tile("zpx", (1, 16))
        nc.vector.memset(zpx[:], 0.0)
        nc.vector.tensor_scalar(zpx[0:1, 2:10], csb[0:1, 0:8],
                                xmaxr[0:1, 0:1], None, OP.mult)
        nc.gpsimd.dma_start(c_mx_i[:, :], zpx[:])
        nc.gpsimd.collective_compute(
            "AllReduce", OP.add, replica_groups=ALL8,
            ins=[c_mx_i[:, :].opt()], outs=[c_mx_o[:, :].opt()])

        # ============ Stage 0b: |W| abs passes -> AR_A, AR_B ===============
        # W_qk slabs also cast to f16 and KEPT in SBUF for in-place quantize.
        accA = sc_tile("accA", (P, 1))
        accB = sc_tile("accB", (P, 1))
        ps0 = ExitStack()
        ps0_pool = ps0.enter_context(tc.tile_pool(name="ps0", bufs=1,
                                                  space="PSUM"))
        absdump = ps0_pool.tile([P, 2048], F32, tag="absdump")

        s1es = ExitStack()
        xq_pool = s1es.enter_context(tc.tile_pool(name="xq", bufs=HB))
        wq_es = ExitStack()
        wq_pool = wq_es.enter_context(tc.tile_pool(name="wq16", bufs=HB))

        wq16 = []
        for ob in range(HB):
            wf = stream.tile([P, 2048], F32, tag="st32")
            nc.sync.dma_start(wf[:], wqkt[ob, :, :])
            r = red.tile([P, 1], F32, tag="wred")
            nc.scalar.activation(absdump[:], wf[:], AF.Abs, accum_out=r[:])
            if ob == 0:
                nc.vector.tensor_copy(accA[:], r[:])
            else:
                nc.vector.tensor_tensor(accA[:], accA[:], r[:], OP.add)
            w16 = wq_pool.tile([P, 2048], F16, tag="wq16")
            nc.vector.tensor_copy(w16[:], wf[:])
            wq16.append(w16)

        for t in range(HB):
            wf = stream2.tile([P, 1024], F32, tag="sv32")
            nc.sync.dma_start(wf[:], wvt[t * P:(t + 1) * P, :])
            r = red.tile([P, 1], F32, tag="wred")
            nc.scalar.activation(absdump[:, 0:1024], wf[:], AF.Abs,
                                 accum_out=r[:])
            nc.vector.tensor_tensor(accA[:], accA[:], r[:], OP.add)

        accAr = sc_tile("accAr", (P, 1))
        nc.gpsimd.partition_all_reduce(accAr[:], accA[:], channels=P,
                                       reduce_op=bass_isa.ReduceOp.add)
        zpa = sc_tile("zpa", (1, 16))
        nc.vector.memset(zpa[:], 0.0)
        nc.vector.tensor_copy(zpa[0:1, 0:1], accAr[0:1, 0:1])
        nc.gpsimd.dma_start(c_add_i[:, :], zpa[:])
        nc.gpsimd.collective_compute(
            "AllReduce", OP.add, replica_groups=ALL8,
            ins=[c_add_i[:, :].opt()], outs=[c_add_o[:, :].opt()])

        for t in range(HALF // P):
            wf = stream.tile([P, 2048], F32, tag="st32")
            nc.sync.dma_start(wf[:], wdt[t * P:(t + 1) * P, :])
            r = red.tile([P, 1], F32, tag="wred")
            nc.scalar.activation(absdump[:], wf[:], AF.Abs, accum_out=r[:])
            if t == 0:
                nc.vector.tensor_copy(accB[:], r[:])
            else:
                nc.vector.tensor_tensor(accB[:], accB[:], r[:], OP.add)
        accBr = sc_tile("accBr", (P, 1))
        nc.gpsimd.partition_all_reduce(accBr[:], accB[:], channels=P,
                                       reduce_op=bass_isa.ReduceOp.add)
        zpb = sc_tile("zpb", (1, 16))
        nc.vector.memset(zpb[:], 0.0)
        nc.vector.tensor_copy(zpb[0:1, 1:2], accBr[0:1, 0:1])
        nc.gpsimd.dma_start(c_wd_i[:, :], zpb[:])
        nc.gpsimd.collective_compute(
            "AllReduce", OP.add, replica_groups=ALL8,
            ins=[c_wd_i[:, :].opt()], outs=[c_wd_o[:, :].opt()])

        ps0.close()

        # ============ scales (x first: xq is on the critical path) =========
        xm = sc_tile("xm", (1, 16))
        nc.sync.dma_start(xm[:], c_mx_o[:, :])
        xmx = sc_tile("xmx")
        nc.vector.tensor_reduce(xmx[:], xm[0:1, 2:10],
                                axis=mybir.AxisListType.X, op=OP.max)
        xm1 = sc_tile("xm1")
        nc.vector.tensor_scalar(xm1[:], xmx[:], 1e-8, None, OP.add)
        rxm = sc_tile("rxm")
        nc.vector.reciprocal(rxm[:], xm1[:])
        sx = sc_tile("sx")
        nc.vector.tensor_scalar(sx[:], rxm[:], 127.0, None, OP.mult)
        sx_b = bcast("sx_b", sx)

        # quantize x: round(x*sx) -> f16 ints, kept in SBUF
        xq = []
        for kb in range(HB):
            xf = stream.tile([P, TOK], F32, tag="st32")
            nc.sync.dma_start(xf[:], xt[kb * P:(kb + 1) * P, :])
            t1 = stream.tile([P, TOK], F32, tag="st32")
            nc.vector.tensor_scalar(t1[:], xf[:], sx_b[:], MAGIC,
                                    OP.mult, OP.add)
            q = xq_pool.tile([P, TOK], F16, tag="xq")
            nc.vector.tensor_scalar(q[:], t1[:], MAGIC, None, OP.subtract)
            xq.append(q)

        # gamma_qkv = sum|W_qkv|/(3H*H)+1e-5 (all-8 add = 4x full sum)
        wsA = sc_tile("wsA", (1, 16))
        nc.sync.dma_start(wsA[:], c_add_o[:, :])
        gq = sc_tile("gq")
        nc.vector.tensor_scalar(gq[:], wsA[0:1, 0:1],
                                1.0 / (4 * 3 * H * H), 1e-5, OP.mult, OP.add)
        igq = sc_tile("igq")
        nc.vector.reciprocal(igq[:], gq[:])
        igq_b = bcast("igq_b", igq)

        # quantize W_qk slabs IN PLACE (f16 magic round, clip to [-1,1])
        for ob in range(HB):
            t1 = stream2.tile([P, 2048], F16, tag="q16")
            nc.vector.tensor_scalar(t1[:], wq16[ob][:], igq_b[:], M16,
                                    OP.mult, OP.add)
            t2 = stream2.tile([P, 2048], F16, tag="q16")
            nc.vector.tensor_scalar(t2[:], t1[:], M16, 1.0,
                                    OP.subtract, OP.min)
            nc.vector.tensor_scalar(wq16[ob][:], t2[:], -1.0, None, OP.max)

        # remaining scales
        al_t = sc_tile("al_t")
        nc.vector.tensor_tensor(al_t[:], gq[:], xm1[:], OP.mult)
        alpha = sc_tile("alpha")
        nc.vector.tensor_scalar(alpha[:], al_t[:], 1.0 / 127.0, None, OP.mult)
        alpha_b = bcast("alpha_b", alpha)
        a2 = sc_tile("a2")
        nc.vector.tensor_tensor(a2[:], alpha[:], alpha[:], OP.mult)
        nc.vector.tensor_scalar(a2[:], a2[:], INV_SQD, None, OP.mult)
        a2_b = bcast("a2_b", a2)
        # sign-route (W_v, W_d) gives {-2,0,2}; alpv/alphad carry the 0.5
        alpv = sc_tile("alpv")
        nc.vector.tensor_scalar(alpv[:], alpha[:], 0.5, None, OP.mult)
        alpv_b = bcast("alpv_b", alpv)
        ntq = sc_tile("ntq")
        nc.vector.tensor_scalar(ntq[:], gq[:], -0.5, None, OP.mult)
        ntq_b = bcast("ntq_b", ntq)
        ptq = sc_tile("ptq")
        nc.vector.tensor_scalar(ptq[:], gq[:], 0.5, None, OP.mult)
        ptq_b = bcast("ptq_b", ptq)

        wsB = sc_tile("wsB", (1, 16))
        nc.sync.dma_start(wsB[:], c_wd_o[:, :])
        gd = sc_tile("gd")
        nc.vector.tensor_scalar(gd[:], wsB[0:1, 1:2],
                                1.0 / (4 * H * H), 1e-5, OP.mult, OP.add)
        ntd = sc_tile("ntd")
        nc.vector.tensor_scalar(ntd[:], gd[:], -0.5, None, OP.mult)
        ntd_b = bcast("ntd_b", ntd)
        ptd = sc_tile("ptd")
        nc.vector.tensor_scalar(ptd[:], gd[:], 0.5, None, OP.mult)
        ptd_b = bcast("ptd_b", ptd)

        bvb = None
        if not v_bias_zero:
            bv_sb = smalls.tile([1, 1024], F32, tag="bv_sb")
            nc.sync.dma_start(bv_sb[:], bv[:, :])
            bvb = smalls.tile([P, 1024], F32, tag="bvb")
            nc.gpsimd.partition_broadcast(bvb[:], bv_sb[:])

        # sign-quantize W_v / W_d on the Scalar engine -> DRAM f16
        def quantize_w_sign(dram_in, dram_out, nrows, width, ntb, ptb):
            for t in range(nrows // P):
                wf = stream2.tile([P, width], F32, tag="sv32")
                nc.sync.dma_start(wf[:], dram_in[t * P:(t + 1) * P, :])
                s1 = stream.tile([P, width], F16, tag="sg16")
                nc.scalar.activation(s1[:], wf[:], AF.Sign, bias=ntb[:])
                s2 = stream.tile([P, width], F16, tag="sg16")
                nc.scalar.activation(s2[:], wf[:], AF.Sign, bias=ptb[:])
                t3 = stream.tile([P, width], F16, tag="sg16")
                nc.vector.tensor_tensor(t3[:], s1[:], s2[:], OP.add)
                nc.sync.dma_start(dram_out[t * P:(t + 1) * P, :], t3[:])

        quantize_w_sign(wvt, wv_q, H, 1024, ntq_b, ptq_b)
        quantize_w_sign(wdt, wd_q, HALF, H, ntd_b, ptd_b)

        # ============ Stage 1: QKV projection ==============================
        with tc.tile_pool(name="s1ev", bufs=3) as ev_pool, \
             tc.tile_pool(name="ps1", bufs=2, space="PSUM") as ps1:
            # Q^T and K^T: weights already in SBUF (wq16 slabs); raw integer
            # sums when bias is zero (alpha^2 folded into exp scale).
            for ob in range(16):
                psum = ps1.tile([P, TOK], F32, tag="ps")
                for kb in range(HB):
                    for sl in _chunks(4, 512):
                        nc.tensor.matmul(psum[:, sl],
                                         lhsT=wq16[ob][:, kb * P:(kb + 1) * P],
                                         rhs=xq[kb][:, sl],
                                         start=(kb == 0), stop=(kb == HB - 1))
                ev = ev_pool.tile([P, TOK], F16, tag="ev")
                if qk_bias_zero:
                    nc.vector.tensor_copy(ev[:], psum[:])
                else:
                    nc.scalar.activation(ev[:], psum[:], AF.Identity,
                                         bias=bqk_sb[:, ob:ob + 1],
                                         scale=alpha_b[:])
                nc.sync.dma_start(qkt_d[ob, :, :], ev[:])
        wq_es.close()

        with tc.tile_pool(name="wv_sb", bufs=HB) as wv_pool, \
             tc.tile_pool(name="evv", bufs=3) as evv_pool, \
             tc.tile_pool(name="ps1v", bufs=2, space="PSUM") as ps1v:
            # V: [tok, 1024] per token block, kept in SBUF bf16 (raw ints
            # when bias zero; alpv folded into the softmax reciprocal).
            wv_list = []
            for kb in range(HB):
                wvq = wv_pool.tile([P, 1024], F16, tag="wv_sb")
                nc.sync.dma_start(wvq[:], wv_q[kb * P:(kb + 1) * P, :])
                wv_list.append(wvq)
            for tb in range(HB):
                psum = ps1v.tile([P, 1024], F32, tag="ps")
                for kb in range(HB):
                    for sl in _chunks(2, 512):
                        nc.tensor.matmul(
                            psum[:, sl],
                            lhsT=xq[kb][:, tb * P:(tb + 1) * P],
                            rhs=wv_list[kb][:, sl],
                            start=(kb == 0), stop=(kb == HB - 1))
                v = evv_pool.tile([P, 1024], BF16, tag="vt")
                if v_bias_zero:
                    nc.vector.tensor_copy(v[:], psum[:])
                else:
                    nc.vector.scalar_tensor_tensor(v[:], psum[:], alpv_b[:],
                                                   bvb[:], OP.mult, OP.add)
                nc.sync.dma_start(vt_d[tb, :, :], v[:])
        s1es.close()

        # ============ Stage 2: attention ===================================
        # The psc eviction is split: a cheap TS (raw ctx * alpv) frees the
        # ctx PSUM immediately; the slow normalize chain (broadcast 1/denom,
        # DVE reciprocal ~6.5us, multiply) is deferred into the NEXT half's
        # emission so it hides under that half's matmuls.  Normalized ctx is
        # kept in SBUF as f16 for stage 3.
        mxacc = sc_tile("mxacc", (P, 1))
        cn_es = ExitStack()
        cn_pool = cn_es.enter_context(tc.tile_pool(name="cn", bufs=16))
        cn_keep = {}
        state = {"first_mx": True, "pend": None}

        def finish_half(p):
            hh, qq, cnr, psd_s = p
            rb = rb_pool.tile([P, 1024], F32, tag="rb")
            nc.gpsimd.partition_broadcast(rb[:], psd_s[:])
            rbr = rb_pool.tile([P, 1024], F32, tag="rb")
            nc.vector.reciprocal(rbr[:], rb[:])
            cnf = cn_pool.tile([P, 1024], F16, tag="cnh")
            nc.vector.tensor_tensor(cnf[:], cnr[:], rbr[:], OP.mult)
            r = red.tile([P, 1], F32, tag="cmax")
            nc.vector.tensor_reduce(r[:], cnf[:], axis=mybir.AxisListType.X,
                                    op=OP.max, apply_absolute_value=True)
            if state["first_mx"]:
                nc.vector.tensor_copy(mxacc[:], r[:])
                state["first_mx"] = False
            else:
                nc.vector.tensor_tensor(mxacc[:], mxacc[:], r[:], OP.max)
            cn_keep[(hh, qq)] = cnf

        with tc.tile_pool(name="qkt", bufs=4) as qk_pool, \
             tc.tile_pool(name="vh", bufs=28) as vh_pool, \
             tc.tile_pool(name="et", bufs=20) as et_pool, \
             tc.tile_pool(name="rb", bufs=2) as rb_pool, \
             tc.tile_pool(name="cnr", bufs=2) as cnr_pool, \
             tc.tile_pool(name="rd", bufs=2) as rd_pool, \
             tc.tile_pool(name="ps2s", bufs=2, space="PSUM") as ps2s, \
             tc.tile_pool(name="ps2c", bufs=1, space="PSUM") as ps2c, \
             tc.tile_pool(name="ps2d", bufs=1, space="PSUM") as ps2d:
            for h in range(NHC):
                qt = qk_pool.tile([P, TOK], F16, tag="qt")
                nc.sync.dma_start(qt[:], qkt_d[h, :, :])
                kt = qk_pool.tile([P, TOK], F16, tag="kt")
                nc.sync.dma_start(kt[:], qkt_d[NHC + h, :, :])
                vh = []
                for kb in range(HB):
                    vk = vh_pool.tile([P, P], BF16, tag="vh")
                    nc.sync.dma_start(vk[:], vt_d[kb, :, h * P:(h + 1) * P])
                    vh.append(vk)

                for qh in range(2):
                    q0 = qh * 1024
                    et = []
                    for kb in range(HB):
                        pss = ps2s.tile([P, 1024], F32, tag="pss")
                        for sl, psl in zip(_chunks(2, 512, q0),
                                           _chunks(2, 512)):
                            nc.tensor.matmul(pss[:, psl],
                                             lhsT=kt[:, kb * P:(kb + 1) * P],
                                             rhs=qt[:, sl],
                                             start=True, stop=True)
                        e = et_pool.tile([P, 1024], BF16, tag="et")
                        nc.scalar.activation(
                            e[:], pss[:], AF.Exp,
                            bias=(mask_sb[:, kb:kb + 1] if use_mask else 0.0),
                            scale=(a2_b[:] if qk_bias_zero else INV_SQD))
                        et.append(e)

                    if state["pend"] is not None:
                        finish_half(state["pend"])
                        state["pend"] = None

                    psc = ps2c.tile([P, 1024], F32, tag="psc")
                    psd = ps2d.tile([1, 1024], F32, tag="psd")
                    for kb in range(HB):
                        vv = vh[kb][:]
                        for sl in _chunks(2, 512):
                            nc.tensor.matmul(psc[:, sl], lhsT=vv,
                                             rhs=et[kb][:, sl],
                                             start=(kb == 0),
                                             stop=(kb == HB - 1))
                        for sl in _chunks(2, 512):
                            nc.tensor.matmul(psd[:, sl], lhsT=ones_col[:],
                                             rhs=et[kb][:, sl],
                                             start=(kb == 0),
                                             stop=(kb == HB - 1))

                    cnr = cnr_pool.tile([P, 1024], F32, tag="cnr")
                    if v_bias_zero:
                        nc.vector.tensor_scalar(cnr[:], psc[:], alpv_b[:],
                                                None, OP.mult)
                    else:
                        nc.vector.tensor_copy(cnr[:], psc[:])
                    psd_s = rd_pool.tile([1, 1024], F32, tag="rd")
                    nc.vector.tensor_copy(psd_s[:], psd[:, :])
                    state["pend"] = (h, qh, cnr, psd_s)
            finish_half(state["pend"])
            state["pend"] = None


        # ============ ctx max AllReduce + quantize scales ==================
        mxr = sc_tile("mxr", (P, 1))
        nc.gpsimd.partition_all_reduce(mxr[:], mxacc[:], channels=P,
                                       reduce_op=bass_isa.ReduceOp.max)
        zpad3 = sc_tile("zpad3", (1, 16))
        nc.vector.memset(zpad3[:], 0.0)
        nc.vector.tensor_copy(zpad3[0:1, 0:1], mxr[0:1, 0:1])
        nc.gpsimd.dma_start(c_mc_i[:, :], zpad3[:])
        nc.gpsimd.collective_compute(
            "AllReduce", OP.max, replica_groups=ALL8,
            ins=[c_mc_i[:, :].opt()], outs=[c_mc_o[:, :].opt()])
        cm = sc_tile("cm", (1, 16))
        nc.sync.dma_start(cm[:], c_mc_o[:, :])

        cm1 = sc_tile("cm1")
        nc.vector.tensor_scalar(cm1[:], cm[0:1, 0:1], 1e-8, None, OP.add)
        rcm = sc_tile("rcm")
        nc.vector.reciprocal(rcm[:], cm1[:])
        sctx = sc_tile("sctx")
        nc.vector.tensor_scalar(sctx[:], rcm[:], 127.0, None, OP.mult)
        ad_t = sc_tile("ad_t")
        nc.vector.tensor_tensor(ad_t[:], gd[:], cm1[:], OP.mult)
        # extra 0.5: W_d was sign-quantized to {-2,0,2}
        alphad = sc_tile("alphad")
        nc.vector.tensor_scalar(alphad[:], ad_t[:], 0.5 / 127.0, None, OP.mult)
        sctx_b = bcast("sctx_b", sctx)
        alphad_b = bcast("alphad_b", alphad)

        bdb = None
        if not d_bias_zero:
            bd_sb = smalls.tile([1, H], F32, tag="bd_sb")
            nc.sync.dma_start(bd_sb[:], bdh[:, :])
            bdb = smalls.tile([P, H], F32, tag="bdb")
            nc.gpsimd.partition_broadcast(bdb[:], bd_sb[:])

        # ============ Stage 3: quantize ctx, dense, chunked RS =============
        with tc.tile_pool(name="cq", bufs=NHC) as cq_pool, \
             tc.tile_pool(name="wd_sb", bufs=NHC) as wd_pool, \
             tc.tile_pool(name="s3ev", bufs=3) as ev3_pool, \
             tc.tile_pool(name="ps3", bufs=2, space="PSUM") as ps3:
            wd_sb = []
            for kb in range(NHC):
                w = wd_pool.tile([P, H], F16, tag="wd_sb")
                nc.sync.dma_start(w[:], wd_q[kb * P:(kb + 1) * P, :])
                wd_sb.append(w)

            ctxq = []
            for h in range(NHC):
                q = cq_pool.tile([P, TOK], F16, tag="cq")
                for qh in range(2):
                    q0 = qh * 1024
                    t1 = stream2.tile([P, 1024], F16, tag="cq16")
                    nc.vector.tensor_scalar(t1[:], cn_keep[(h, qh)][:],
                                            sctx_b[:], M16, OP.mult, OP.add)
                    nc.vector.tensor_scalar(q[:, q0:q0 + 1024], t1[:], M16,
                                            None, OP.subtract)
                ctxq.append(q)

            # token-interleaved chunks: RS chunk j fires after its 4 token
            # blocks, overlapping the remaining dense matmuls; LN follows
            # per chunk.
            for j in range(4):
                for tb in TBORD[4 * j:4 * j + 4]:
                    psum = ps3.tile([P, TOK], F32, tag="ps")
                    for kb in range(NHC):
                        for sl in _chunks(4, 512):
                            nc.tensor.matmul(
                                psum[:, sl],
                                lhsT=ctxq[kb][:, tb * P:(tb + 1) * P],
                                rhs=wd_sb[kb][:, sl],
                                start=(kb == 0), stop=(kb == NHC - 1))
                    ev = ev3_pool.tile([P, TOK], BF16, tag="ev3")
                    if d_bias_zero:
                        # raw int sums; alphad folded into LN input scale
                        nc.vector.tensor_copy(ev[:], psum[:])
                    else:
                        nc.vector.scalar_tensor_tensor(
                            ev[:], psum[:], alphad_b[:], bdb[:],
                            OP.mult, OP.add)
                    off = (0 if tb < 8 else 256) + (tb % 2) * P
                    nc.sync.dma_start(rs_in[j, off:off + P, :], ev[:])
                nc.gpsimd.collective_compute(
                    "ReduceScatter", OP.add, replica_groups=PAIRS,
                    ins=[rs_in[j, :, :].opt()], outs=[rs_out[j, :, :].opt()])
        cn_es.close()

        lnwb = lnbb = None
        if not ln_trivial:
            lnw_sb = smalls.tile([1, H], F32, tag="lnw_sb")
            nc.sync.dma_start(lnw_sb[:], lnw[:, :])
            lnwb = smalls.tile([P, H], F32, tag="lnwb")
            nc.gpsimd.partition_broadcast(lnwb[:], lnw_sb[:])
            lnb_sb = smalls.tile([1, H], F32, tag="lnb_sb")
            nc.sync.dma_start(lnb_sb[:], lnb[:, :])
            lnbb = smalls.tile([P, H], F32, tag="lnbb")
            nc.gpsimd.partition_broadcast(lnbb[:], lnb_sb[:])

        # ============ Stage 4: residual + layernorm ========================
        with tc.tile_pool(name="ln", bufs=2) as ln_pool, \
             tc.tile_pool(name="lns", bufs=4) as lns_pool:
            for m in range(HALF // P):
                r_t = ln_pool.tile([P, H], BF16, tag="lnr")
                nc.sync.dma_start(r_t[:],
                                  rs_out[m // 2, (m % 2) * P:(m % 2) * P + P, :])
                x_t = ln_pool.tile([P, H], F32, tag="lnx")
                nc.sync.dma_start(x_t[:], xr[m * P:(m + 1) * P, :])

                y = ln_pool.tile([P, H], F32, tag="lny")
                ysum = lns_pool.tile([P, 1], F32, tag="ysum")
                dscale = alphad_b[:] if d_bias_zero else 1.0
                nc.vector.scalar_tensor_tensor(y[:], r_t[:], dscale, x_t[:],
                                               OP.mult, OP.add,
                                               accum_out=ysum[:])
                mu = lns_pool.tile([P, 1], F32, tag="mu")
                nc.vector.tensor_scalar(mu[:], ysum[:], 1.0 / H, None, OP.mult)
                nmu = lns_pool.tile([P, 1], F32, tag="nmu")
                nc.vector.tensor_scalar(nmu[:], mu[:], -1.0, None, OP.mult)

                sq = ln_pool.tile([P, H], F32, tag="lnsq")
                sqs = lns_pool.tile([P, 1], F32, tag="sqs")
                nc.scalar.activation(sq[:], y[:], AF.Square,
                                     bias=nmu[:], scale=1.0,
                                     accum_out=sqs[:])
                v1 = lns_pool.tile([P, 1], F32, tag="v1")
                nc.vector.tensor_scalar(v1[:], sqs[:], 1.0 / H, LN_EPS,
                                        OP.mult, OP.add)
                v2 = lns_pool.tile([P, 1], F32, tag="v2")
                nc.vector.reciprocal(v2[:], v1[:])
                rstd = lns_pool.tile([P, 1], F32, tag="rstd")
                nc.scalar.activation(rstd[:], v2[:], AF.Sqrt)
                nmr = lns_pool.tile([P, 1], F32, tag="nmr")
                nc.vector.tensor_tensor(nmr[:], nmu[:], rstd[:], OP.mult)

                yn = ln_pool.tile([P, H], F32, tag="lnyn")
                nc.scalar.activation(yn[:], y[:], AF.Identity,
                                     bias=nmr[:], scale=rstd[:])
                if not ln_trivial:
                    nc.vector.tensor_tensor(yn[:], yn[:], lnwb[:], OP.mult)
                    nc.vector.tensor_tensor(yn[:], yn[:], lnbb[:], OP.add)
                nc.sync.dma_start(out[m * P:(m + 1) * P, :], yn[:])


# ======================= host side =======================================

def make_in_maps(hidden_states, attention_mask, W_qkv, b_qkv, W_dense,
                 b_dense, ln_w, ln_b):
    x = np.asarray(hidden_states, dtype=np.float32)
    mask = np.asarray(attention_mask, dtype=np.float32)
    Wq = np.asarray(W_qkv, dtype=np.float32)
    bq = np.asarray(b_qkv, dtype=np.float32)
    Wd = np.asarray(W_dense, dtype=np.float32)
    bd = np.asarray(b_dense, dtype=np.float32)
    lw = np.asarray(ln_w, dtype=np.float32)
    lb = np.asarray(ln_b, dtype=np.float32)

    in_maps = []
    for c in range(NCORES):
        b, g = c // 2, c % 2
        sl = slice(g * 1024, (g + 1) * 1024)
        wq_g = Wq[sl, :]
        wk_g = Wq[2048 + g * 1024:2048 + (g + 1) * 1024, :]
        wv_g = Wq[4096 + g * 1024:4096 + (g + 1) * 1024, :]
        bq_g = bq[sl]
        bk_g = bq[2048 + g * 1024:2048 + (g + 1) * 1024]
        bv_g = bq[4096 + g * 1024:4096 + (g + 1) * 1024]
        W2 = np.concatenate([wq_g, wk_g], axis=0).T  # [h, out]
        wq_tiled = np.ascontiguousarray(
            W2.reshape(16, P, 16, P).transpose(2, 1, 0, 3).reshape(16, P, H))
        in_maps.append({
            "xt": np.ascontiguousarray(x[b].T),
            "xr": np.ascontiguousarray(x[b, g * 1024:(g + 1) * 1024, :]),
            "wqkt": wq_tiled,
            "wvt": np.ascontiguousarray(wv_g.T),
            "bqk": np.ascontiguousarray(
                np.concatenate([bq_g, bk_g]).reshape(16, P).T),
            "bv": bv_g.reshape(1, 1024).copy(),
            "wdt": np.ascontiguousarray(Wd[:, g * 1024:(g + 1) * 1024].T),
            "bdh": (bd * 0.5).reshape(1, H).copy(),
            "maskt": np.ascontiguousarray(mask[b, 0, 0, :].reshape(HB, P).T),
            "csel": np.eye(16, dtype=np.float32)[2 + c].reshape(1, 16).copy(),
            "lnw": lw.reshape(1, H).copy(),
            "lnb": lb.reshape(1, H).copy(),
        })
    return in_maps


def build_flags(attention_mask, b_qkv, b_dense, ln_w, ln_b):
    return (
        bool(np.any(np.asarray(attention_mask) != 0.0)),
        bool(np.all(np.asarray(b_qkv)[:4096] == 0.0)),
        bool(np.all(np.asarray(b_qkv)[4096:] == 0.0)),
        bool(np.all(np.asarray(b_dense) == 0.0)),
        bool(np.all(np.asarray(ln_w) == 1.0) and np.all(np.asarray(ln_b) == 0.0)),
    )


def assemble_output(results):
    full = np.empty((B, S, H), dtype=np.float32)
    for c in range(NCORES):
        b, g = c // 2, c % 2
        full[b, g * 1024:(g + 1) * 1024, :] = results[c]["out"]
    return full


_CACHE = {}


def _get_program(flags):
    if flags not in _CACHE:
        _CACHE[flags] = build_program(*flags)
    return _CACHE[flags]


def _ensure_ntff_hook():
    """Provide antenv.axon_hooks (missing in this image) so trace=True can
    capture NTFF profiles through the axon PJRT plugin."""
    import types

    try:
        import antenv.axon_hooks  # noqa: F401
        return
    except ImportError:
        pass
    try:
        import antenv
    except ImportError:
        return
    mod = types.ModuleType("antenv.axon_hooks")
    holder = {"h": None}
    mod.set_axon_ntff_profile_hook = lambda h: holder.__setitem__("h", h)
    mod.get_axon_ntff_profile_hook = lambda: holder["h"]
    sys.modules["antenv.axon_hooks"] = mod
    antenv.axon_hooks = mod
    try:
        if "/root/.axon_site" not in sys.path:
            sys.path.insert(0, "/root/.axon_site")
        from trn_agent_boot.trn_boot import _ntff_profile_via_ctypes
        h = _ntff_profile_via_ctypes("/opt/axon/libaxon_pjrt.so")
        if h is not None:
            mod.set_axon_ntff_profile_hook(h)
    except Exception:
        pass


def kernel(hidden_states, attention_mask, W_qkv, b_qkv, W_dense, b_dense,
           ln_w, ln_b, trace=False):
    from concourse.bass_utils import run_bass_kernel_spmd

    flags = build_flags(attention_mask, b_qkv, b_dense, ln_w, ln_b)
    nc = _get_program(flags)
    in_maps = make_in_maps(hidden_states, attention_mask, W_qkv, b_qkv,
                           W_dense, b_dense, ln_w, ln_b)
    if trace:
        _ensure_ntff_hook()
        try:
            res = run_bass_kernel_spmd(nc, in_maps,
                                       core_ids=list(range(NCORES)),
                                       trace=True)
        except Exception as e:
            print("trace run failed (%s); retrying untraced" % e)
            res = run_bass_kernel_spmd(nc, in_maps,
                                       core_ids=list(range(NCORES)),
                                       trace=False)
    else:
        res = run_bass_kernel_spmd(nc, in_maps, core_ids=list(range(NCORES)),
                                   trace=False)
    out = assemble_output(res.results)
    kernel.last_result = res
    return out
